# revision 13
# baseline (speedup 1.0000x reference)
"""Trainium2 Bass kernel for nn_Expert (gather-span + 2-layer linear MLP).

Reference computation (B=32, L=4096, H=1024, N=4):
    idx      = pos + arange(N)                      # (B, N)
    gathered = hidden[b, idx[b, n], :]              # (B, N, H)
    x        = gathered.reshape(B, N*H)             # (B, 4096)
    out      = (x @ W1.T + b1) @ W2.T + b2          # (B, 4)

Sharding (8 cores): the contraction dim (N*H = 4096) is split across
cores -- hidden sharded on its last dim in 128-wide slices, W1 sharded
over the matching contraction columns.  Each core reads 2 MB of W1 (the
dominant, irreducible HBM traffic; the problem is memory-bound) plus a
64 KB gathered slice of hidden, computes partial out1/(N,B) y partials,
and the host sums the 8 partials.

Schedule notes (from profiling on this part):
  * The indirect (SWDGE) gather's packets only run when the HWDGE
    rings are idle -- they do NOT round-robin with an active W1
    stream.  So the gather goes FIRST: its descriptors are generated
    while only tiny transfers are in flight, and the sync-ring W1
    pieces are released only after descriptor generation (enforced by
    a WAW dep: tiny gpsimd memsets into the W1 tiles, ordered after
    the indirect DMA).  The scalar ring streams immediately -- the
    gather slots in after its first 256 KB batch.
  * The gather is 32x 2KB descriptors: one per batch row, exploiting
    that the N=4 span rows are contiguous in (B*L, 128) layout.
  * fp32 matmuls run LOW+HIGH passes at the HAM-cold 1.2 GHz rate
    (~430ns per 512-col pass); stage 1 is column-tiled (group A ->
    psum partitions 0:32 = out1[:, 0:512], group B -> 32:64, col_grp
    q32) so the two groups' passes run concurrently and PE stays
    under the DMA rate.
  * b1 enters PSUM via K=1 matmuls (group start) while the PE is
    otherwise idle; b2 rides a (4,32) pre-broadcast add at the end.
  * Stage 2 is straight-line batches (no PE<->DVE ping-pong): copy
    out1 to SBUF (vector || scalar), 8 PE transposes alternating row
    strips q0/q32, one copy, 8 accumulating y matmuls.
The host sums per-core y partials (fp64) and transposes to (B, N).
"""

import numpy as np

from concourse import bass, bacc, mybir
from concourse.tile import TileContext
from concourse.bass_utils import run_bass_kernel_spmd
from concourse.masks import make_identity

B, L, H, N = 32, 4096, 1024, 4
NCORES = 8
HS = H // NCORES       # 128: per-core slice of the hidden dim
P = 128
HB = H // 2            # 512: psum bank width for stage 1
F32 = mybir.dt.float32
I32 = mybir.dt.int32

TRACE = False          # set True in test harnesses to profile
LAST_EXEC_NS = None

_nc_cache = None


def _build_nc():
    nc = bacc.Bacc(target_bir_lowering=False)
    hid = nc.declare_dram_parameter("hid", [B * L, HS], F32, isOutput=False)
    idxd = nc.declare_dram_parameter("idxd", [1, B], I32, isOutput=False)
    w1t = nc.declare_dram_parameter("w1t", [N * P, H], F32, isOutput=False)
    w2d = nc.declare_dram_parameter("w2d", [P, 2 * N * N], F32, isOutput=False)
    b1d = nc.declare_dram_parameter("b1d", [1, H], F32, isOutput=False)
    b2d = nc.declare_dram_parameter("b2d", [N, B], F32, isOutput=False)
    outd = nc.declare_dram_parameter("out", [N, B], F32, isOutput=True)

    with TileContext(nc) as tc:
        with (
            tc.tile_pool(name="sbuf", bufs=1) as spool,
            tc.tile_pool(name="ps1", bufs=1, space="PSUM") as pool1,
            tc.tile_pool(name="psX", bufs=1, space="PSUM") as poolX,
            tc.tile_pool(name="psT", bufs=1, space="PSUM") as poolT,
            tc.tile_pool(name="psY", bufs=1, space="PSUM") as poolY,
        ):
            w1sb = [
                spool.tile([P, H], F32, tag=f"w1_{n}", name=f"w1_{n}")
                for n in range(N)
            ]

            def w1piece(n, h):
                eng = nc.sync if n in (0, 1) else nc.scalar
                eng.dma_start(
                    out=w1sb[n][:, h * HB:(h + 1) * HB],
                    in_=w1t[n * P:(n + 1) * P, h * HB:(h + 1) * HB],
                )

            # sync ring: gather indices first
            idxi = spool.tile([B, 1], I32)
            nc.sync.dma_start(out=idxi[:], in_=idxd[:])
            # scalar ring: the small late-stage operands go first (they are
            # tiny, and issuing them behind the W1 stream would stall the PE
            # queue on their semaphores mid-stage-1), then b1.
            w2sb = spool.tile([P, 2 * N * N], F32)
            nc.scalar.dma_start(out=w2sb[:], in_=w2d[:])
            b2sb = spool.tile([N, B], F32)
            nc.scalar.dma_start(out=b2sb[:], in_=b2d[:])
            b1sb = spool.tile([1, H], F32)
            nc.scalar.dma_start(out=b1sb[:], in_=b1d[:])

            # ---- identity for PE transposes; ones row for the b1 matmul
            ident = spool.tile([P, P], F32)
            make_identity(nc, ident[:])
            ones1 = spool.tile([1, B], F32)
            nc.vector.memset(ones1[:], 1.0)

            # ---- gather: 32 indices, 4 contiguous 512B rows per index
            xg = spool.tile([B, N * HS], F32)
            nc.gpsimd.indirect_dma_start(
                out=xg[:, :],
                out_offset=None,
                in_=hid[:],
                in_offset=bass.IndirectOffsetOnAxis(ap=idxi[:, :1], axis=0),
                bounds_check=B * L - 1,
                oob_is_err=False,
            )
            # W1 release ops: each ring's first W1 DMA overwrites a corner
            # its release op writes (WAW), so the ring FIFO holds ALL W1
            # pieces until the release runs.  The release ops READ idxi, so
            # the scheduler cannot hoist them before the indirect_dma above
            # (same readiness, later priority) -- at runtime the gpsimd FIFO
            # executes them right after gather descriptor generation.  Net:
            # the gather's SWDGE packets (which starve under active HWDGE
            # traffic) run in the window before the W1 stream starts.
            for tile_i in (0, 2):
                nc.gpsimd.tensor_tensor(
                    out=w1sb[tile_i][0:1, 0:1].bitcast(I32),
                    in0=idxi[0:1, :], in1=idxi[0:1, :],
                    op=mybir.AluOpType.add,
                )

            # W1 pieces (gated per the WAW above)
            for h in range(2):
                w1piece(0, h)
                w1piece(2, h)
            for h in range(2):
                w1piece(1, h)
                w1piece(3, h)

            # ---- stage 1, column-tiled: A = psum rows 0:32 (out1 cols
            # 0:512), B = psum rows 32:64 (out1 cols 512:1024).
            psAB = pool1.tile([2 * B, HB], F32, space="PSUM", tag="ps_ab")

            def mm1(grp, lhsT, rhs, start, stop):
                nc.tensor.matmul(
                    out=psAB[grp * B:(grp + 1) * B, :],
                    lhsT=lhsT, rhs=rhs,
                    start=start, stop=stop, skip_group_check=True,
                )

            mm1(0, ones1[:], b1sb[:1, 0:HB], True, False)
            mm1(1, ones1[:], b1sb[:1, HB:H], True, False)

            # transpose the gather per span-offset chunk, in the order the
            # matmuls consume them: xT[k, n*32+b] = xg[b, n*128+k]
            xtp = poolX.tile([P, P], F32, space="PSUM", tag="xtp")
            xT = spool.tile([P, P], F32)
            for n in (2, 0, 3, 1):
                nc.tensor.transpose(
                    out=xtp[:, n * B:(n + 1) * B],
                    in_=xg[:, n * HS:(n + 1) * HS],
                    identity=ident[:B, :B],
                )
                nc.vector.tensor_copy(
                    out=xT[:, n * B:(n + 1) * B],
                    in_=xtp[:, n * B:(n + 1) * B],
                )

            # matmuls ordered by expected piece arrival (scalar ring first)
            for n, grp, stop in (
                (2, 0, False), (2, 1, False), (0, 0, False), (0, 1, False),
                (3, 0, False), (3, 1, False), (1, 0, True), (1, 1, True),
            ):
                mm1(
                    grp,
                    xT[:, n * B:(n + 1) * B],
                    w1sb[n][:, grp * HB:(grp + 1) * HB],
                    False, stop,
                )

            # ---- stage 2: y[t, b] = b2[t] + sum_o W2[t, o] out1[b, o]
            o1sb = spool.tile([2 * B, HB], F32)
            nc.vector.tensor_copy(out=o1sb[0:B, :], in_=psAB[0:B, :])
            nc.scalar.copy(out=o1sb[B:2 * B, :], in_=psAB[B:2 * B, :])

            trp = poolT.tile([P, 2 * P], F32, space="PSUM", tag="trp")
            trsb = spool.tile([P, 2 * P], F32)
            yps = poolY.tile([N, B], F32, space="PSUM", tag="yps")
            NC = H // P  # 8 chunks of the hidden dim; 0-3 from A, 4-7 from B
            # all transposes back-to-back
            for c in (0, 1, 2, 3, 4, 5, 6, 7):
                g = c // 4
                nc.tensor.transpose(
                    out=trp[:, c * B:(c + 1) * B],
                    in_=o1sb[g * B:(g + 1) * B, (c % 4) * P:(c % 4 + 1) * P],
                    identity=ident[g * B:(g + 1) * B, g * B:(g + 1) * B],
                )
            nc.vector.tensor_copy(out=trsb[:], in_=trp[:])
            for c in range(NC):
                nc.tensor.matmul(
                    out=yps[:],
                    lhsT=w2sb[:, c * N:(c + 1) * N],
                    rhs=trsb[:, c * B:(c + 1) * B],
                    start=(c == 0), stop=(c == NC - 1),
                )

            ysb = spool.tile([N, B], F32)
            nc.vector.tensor_tensor(
                out=ysb[:], in0=yps[:], in1=b2sb[:], op=mybir.AluOpType.add
            )
            nc.sync.dma_start(out=outd[:], in_=ysb[:])

    nc.finalize()
    return nc


def _get_nc():
    global _nc_cache
    if _nc_cache is None:
        _nc_cache = _build_nc()
    return _nc_cache


def kernel(hidden, pos, W1, b1, W2, b2):
    global LAST_EXEC_NS
    hidden = np.asarray(hidden, dtype=np.float32)
    pos = np.asarray(pos)
    W1 = np.asarray(W1, dtype=np.float32)
    b1 = np.asarray(b1, dtype=np.float32)
    W2 = np.asarray(W2, dtype=np.float32)
    b2 = np.asarray(b2, dtype=np.float32)

    # gather row index per batch in the per-core (B*L, HS) layout;
    # the N=4 span rows are contiguous: idxd[b] = b*L + pos[b]
    idxd = (
        np.arange(B, dtype=np.int64) * L + pos.reshape(B).astype(np.int64)
    ).astype(np.int32)[None, :]

    # W1 (H, N*H) -> per-core (N*P, H): w1t_j[n*P+k, o] = W1[o, n*H+j*HS+k]
    w1r = W1.reshape(H, N, NCORES, HS)                 # [o, n, j, k]
    # W2 (N, H) -> (128, 32): w2d[k2, c*N+t] = W2[t, c*P+k2]
    w2d = np.ascontiguousarray(
        W2.reshape(N, H // P, P).transpose(2, 1, 0).reshape(P, -1)
    )
    b1r = b1[None, :]                                  # (1, H), core 0 only
    b1z = np.zeros((1, H), np.float32)
    b2r = np.ascontiguousarray(
        np.broadcast_to(b2[:, None], (N, B))
    )                                                  # (4, 32), core 0 only
    b2z = np.zeros((N, B), np.float32)

    in_maps = []
    for j in range(NCORES):
        hid_j = np.ascontiguousarray(
            hidden[:, :, j * HS:(j + 1) * HS]
        ).reshape(B * L, HS)
        w1t_j = np.ascontiguousarray(
            w1r[:, :, j, :].transpose(1, 2, 0).reshape(N * P, H)
        )
        in_maps.append(
            {
                "hid": hid_j,
                "idxd": idxd,
                "w1t": w1t_j,
                "w2d": w2d,
                "b1d": b1r if j == 0 else b1z,
                "b2d": b2r if j == 0 else b2z,
            }
        )

    nc = _get_nc()
    res = run_bass_kernel_spmd(nc, in_maps, list(range(NCORES)), trace=TRACE)
    LAST_EXEC_NS = res.exec_time_ns

    parts = np.stack([res.results[j]["out"] for j in range(NCORES)])  # (8,4,32)
    ytb = parts.sum(axis=0, dtype=np.float64)                         # (4, 32)
    return np.ascontiguousarray(ytb.T.astype(np.float32))             # (B, N)


# revision 16
# speedup vs baseline: 1.0718x; 1.0718x over previous
"""Trainium2 Bass kernel for nn_Expert (gather-span + 2-layer linear MLP).

Reference computation (B=32, L=4096, H=1024, N=4):
    idx      = pos + arange(N)                      # (B, N)
    gathered = hidden[b, idx[b, n], :]              # (B, N, H)
    x        = gathered.reshape(B, N*H)             # (B, 4096)
    out      = (x @ W1.T + b1) @ W2.T + b2          # (B, 4)

Sharding (8 cores): the contraction dim (N*H = 4096) is split across
cores -- hidden sharded on its last dim in 128-wide slices, W1 sharded
over the matching contraction columns.  Each core reads 2 MB of W1 (the
dominant, irreducible HBM traffic; the problem is memory-bound) plus a
64 KB gathered slice of hidden, computes partial out1/(N,B) y partials,
and the host sums the 8 partials.

Schedule notes (from profiling on this part):
  * The indirect (SWDGE) gather's packets only run when the HWDGE
    rings are idle -- they do NOT round-robin with an active W1
    stream.  So the gather goes FIRST: its descriptors are generated
    while only tiny transfers are in flight, and the sync-ring W1
    pieces are released only after descriptor generation (enforced by
    a WAW dep: tiny gpsimd memsets into the W1 tiles, ordered after
    the indirect DMA).  The scalar ring streams immediately -- the
    gather slots in after its first 256 KB batch.
  * The gather is 32x 2KB descriptors: one per batch row, exploiting
    that the N=4 span rows are contiguous in (B*L, 128) layout.
  * fp32 matmuls run LOW+HIGH passes at the HAM-cold 1.2 GHz rate
    (~430ns per 512-col pass); stage 1 is column-tiled (group A ->
    psum partitions 0:32 = out1[:, 0:512], group B -> 32:64, col_grp
    q32) so the two groups' passes run concurrently and PE stays
    under the DMA rate.
  * b1 enters PSUM via K=1 matmuls (group start) while the PE is
    otherwise idle; b2 rides a (4,32) pre-broadcast add at the end.
  * Stage 2 is straight-line batches (no PE<->DVE ping-pong): copy
    out1 to SBUF (vector || scalar), 8 PE transposes alternating row
    strips q0/q32, one copy, 8 accumulating y matmuls.
The host sums per-core y partials (fp64) and transposes to (B, N).
"""

import numpy as np

from concourse import bass, bacc, mybir
from concourse.tile import TileContext
from concourse.bass_utils import run_bass_kernel_spmd
from concourse.masks import make_identity

B, L, H, N = 32, 4096, 1024, 4
NCORES = 8
HS = H // NCORES       # 128: per-core slice of the hidden dim
P = 128
HB = H // 2            # 512: psum bank width for stage 1
F32 = mybir.dt.float32
I32 = mybir.dt.int32

TRACE = False          # set True in test harnesses to profile
LAST_EXEC_NS = None

_nc_cache = None


def _build_nc():
    nc = bacc.Bacc(target_bir_lowering=False)
    hid = nc.declare_dram_parameter("hid", [B * L, HS], F32, isOutput=False)
    idxd = nc.declare_dram_parameter("idxd", [1, B], I32, isOutput=False)
    w1t = nc.declare_dram_parameter("w1t", [N * P, H], F32, isOutput=False)
    w2d = nc.declare_dram_parameter("w2d", [P, 2 * N * N], F32, isOutput=False)
    b1d = nc.declare_dram_parameter("b1d", [1, H], F32, isOutput=False)
    b2d = nc.declare_dram_parameter("b2d", [N, B], F32, isOutput=False)
    outd = nc.declare_dram_parameter("out", [N, B], F32, isOutput=True)

    with TileContext(nc) as tc:
        with (
            tc.tile_pool(name="sbuf", bufs=1) as spool,
            tc.tile_pool(name="ps1", bufs=1, space="PSUM") as pool1,
            tc.tile_pool(name="psX", bufs=1, space="PSUM") as poolX,
            tc.tile_pool(name="psT", bufs=1, space="PSUM") as poolT,
            tc.tile_pool(name="psY", bufs=1, space="PSUM") as poolY,
        ):
            w1sb = [
                spool.tile([P, H], F32, tag=f"w1_{n}", name=f"w1_{n}")
                for n in range(N)
            ]

            def w1piece(n, h):
                eng = nc.sync if n in (0, 1) else nc.scalar
                eng.dma_start(
                    out=w1sb[n][:, h * HB:(h + 1) * HB],
                    in_=w1t[n * P:(n + 1) * P, h * HB:(h + 1) * HB],
                )

            # gather indices via SWDGE on the gpsimd queue: keeps the whole
            # gather chain (idx load -> indirect gather) on one engine with
            # no HWDGE round-trips, shaving ~2us of first-DMA latency
            idxi = spool.tile([B, 1], I32)
            nc.gpsimd.dma_start(out=idxi[:], in_=idxd[:])
            # scalar ring: the small late-stage operands go first (they are
            # tiny, and issuing them behind the W1 stream would stall the PE
            # queue on their semaphores mid-stage-1), then b1.
            w2sb = spool.tile([P, 2 * N * N], F32)
            nc.scalar.dma_start(out=w2sb[:], in_=w2d[:])
            b2sb = spool.tile([N, B], F32)
            nc.scalar.dma_start(out=b2sb[:], in_=b2d[:])
            b1sb = spool.tile([1, H], F32)
            nc.scalar.dma_start(out=b1sb[:], in_=b1d[:])

            # ---- identity for PE transposes; ones row for the b1 matmul
            ident = spool.tile([P, P], F32)
            make_identity(nc, ident[:])
            ones1 = spool.tile([1, B], F32)
            nc.vector.memset(ones1[:], 1.0)

            # ---- gather: 32 indices, 4 contiguous 512B rows per index
            xg = spool.tile([B, N * HS], F32)
            nc.gpsimd.indirect_dma_start(
                out=xg[:, :],
                out_offset=None,
                in_=hid[:],
                in_offset=bass.IndirectOffsetOnAxis(ap=idxi[:, :1], axis=0),
                bounds_check=B * L - 1,
                oob_is_err=False,
            )
            # W1 release op: the sync ring's first W1 DMA overwrites a corner
            # this op writes (WAW), so the sync-ring FIFO holds its W1 pieces
            # until the release runs.  The release READS idxi, so the
            # scheduler cannot hoist it before the indirect_dma above (same
            # readiness, later priority) -- at runtime the gpsimd FIFO
            # executes it right after gather descriptor generation.  Net:
            # only one HWDGE ring (scalar) contends with the gather's
            # latency-bound random reads; the second joins once the gather
            # descriptors are in flight.
            nc.gpsimd.tensor_tensor(
                out=w1sb[0][0:1, 0:1].bitcast(I32),
                in0=idxi[0:1, :], in1=idxi[0:1, :],
                op=mybir.AluOpType.add,
            )

            # W1 pieces: scalar ring (tiles 2/3) streams immediately; sync
            # ring (tiles 0/1) is gated per the WAW above.
            for h in range(2):
                w1piece(2, h)
                w1piece(0, h)
            for h in range(2):
                w1piece(3, h)
                w1piece(1, h)

            # ---- stage 1, column-tiled: A = psum rows 0:32 (out1 cols
            # 0:512), B = psum rows 32:64 (out1 cols 512:1024).
            psAB = pool1.tile([2 * B, HB], F32, space="PSUM", tag="ps_ab")

            def mm1(grp, lhsT, rhs, start, stop):
                nc.tensor.matmul(
                    out=psAB[grp * B:(grp + 1) * B, :],
                    lhsT=lhsT, rhs=rhs,
                    start=start, stop=stop, skip_group_check=True,
                )

            mm1(0, ones1[:], b1sb[:1, 0:HB], True, False)
            mm1(1, ones1[:], b1sb[:1, HB:H], True, False)

            # transpose the gather (all four back-to-back, then one copy --
            # per-chunk copies invite PE<->DVE ping-pong serialization):
            # xT[k, n*32+b] = xg[b, n*128+k]
            xtp = poolX.tile([P, P], F32, space="PSUM", tag="xtp")
            xT = spool.tile([P, P], F32)
            for n in range(N):
                nc.tensor.transpose(
                    out=xtp[:, n * B:(n + 1) * B],
                    in_=xg[:, n * HS:(n + 1) * HS],
                    identity=ident[:B, :B],
                )
            nc.vector.tensor_copy(out=xT[:], in_=xtp[:])

            # matmuls ordered by expected piece arrival (scalar ring first)
            for n, grp, stop in (
                (2, 0, False), (2, 1, False), (3, 0, False), (3, 1, False),
                (0, 0, False), (0, 1, False), (1, 0, True), (1, 1, True),
            ):
                mm1(
                    grp,
                    xT[:, n * B:(n + 1) * B],
                    w1sb[n][:, grp * HB:(grp + 1) * HB],
                    False, stop,
                )

            # ---- stage 2: y[t, b] = b2[t] + sum_o W2[t, o] out1[b, o]
            o1sb = spool.tile([2 * B, HB], F32)
            nc.vector.tensor_copy(out=o1sb[0:B, :], in_=psAB[0:B, :])
            nc.scalar.copy(out=o1sb[B:2 * B, :], in_=psAB[B:2 * B, :])

            trp = poolT.tile([P, 2 * P], F32, space="PSUM", tag="trp")
            trsb = spool.tile([P, 2 * P], F32)
            yps = poolY.tile([N, B], F32, space="PSUM", tag="yps")
            NC = H // P  # 8 chunks of the hidden dim; 0-3 from A, 4-7 from B
            # all transposes back-to-back
            for c in (0, 1, 2, 3, 4, 5, 6, 7):
                g = c // 4
                nc.tensor.transpose(
                    out=trp[:, c * B:(c + 1) * B],
                    in_=o1sb[g * B:(g + 1) * B, (c % 4) * P:(c % 4 + 1) * P],
                    identity=ident[g * B:(g + 1) * B, g * B:(g + 1) * B],
                )
            nc.vector.tensor_copy(out=trsb[:], in_=trp[:])
            for c in range(NC):
                nc.tensor.matmul(
                    out=yps[:],
                    lhsT=w2sb[:, c * N:(c + 1) * N],
                    rhs=trsb[:, c * B:(c + 1) * B],
                    start=(c == 0), stop=(c == NC - 1),
                )

            ysb = spool.tile([N, B], F32)
            nc.vector.tensor_tensor(
                out=ysb[:], in0=yps[:], in1=b2sb[:], op=mybir.AluOpType.add
            )
            nc.sync.dma_start(out=outd[:], in_=ysb[:])

    nc.finalize()
    return nc


def _get_nc():
    global _nc_cache
    if _nc_cache is None:
        _nc_cache = _build_nc()
    return _nc_cache


def kernel(hidden, pos, W1, b1, W2, b2):
    global LAST_EXEC_NS
    hidden = np.asarray(hidden, dtype=np.float32)
    pos = np.asarray(pos)
    W1 = np.asarray(W1, dtype=np.float32)
    b1 = np.asarray(b1, dtype=np.float32)
    W2 = np.asarray(W2, dtype=np.float32)
    b2 = np.asarray(b2, dtype=np.float32)

    # gather row index per batch in the per-core (B*L, HS) layout;
    # the N=4 span rows are contiguous: idxd[b] = b*L + pos[b]
    idxd = (
        np.arange(B, dtype=np.int64) * L + pos.reshape(B).astype(np.int64)
    ).astype(np.int32)[None, :]

    # W1 (H, N*H) -> per-core (N*P, H): w1t_j[n*P+k, o] = W1[o, n*H+j*HS+k]
    w1r = W1.reshape(H, N, NCORES, HS)                 # [o, n, j, k]
    # W2 (N, H) -> (128, 32): w2d[k2, c*N+t] = W2[t, c*P+k2]
    w2d = np.ascontiguousarray(
        W2.reshape(N, H // P, P).transpose(2, 1, 0).reshape(P, -1)
    )
    b1r = b1[None, :]                                  # (1, H), core 0 only
    b1z = np.zeros((1, H), np.float32)
    b2r = np.ascontiguousarray(
        np.broadcast_to(b2[:, None], (N, B))
    )                                                  # (4, 32), core 0 only
    b2z = np.zeros((N, B), np.float32)

    in_maps = []
    for j in range(NCORES):
        hid_j = np.ascontiguousarray(
            hidden[:, :, j * HS:(j + 1) * HS]
        ).reshape(B * L, HS)
        w1t_j = np.ascontiguousarray(
            w1r[:, :, j, :].transpose(1, 2, 0).reshape(N * P, H)
        )
        in_maps.append(
            {
                "hid": hid_j,
                "idxd": idxd,
                "w1t": w1t_j,
                "w2d": w2d,
                "b1d": b1r if j == 0 else b1z,
                "b2d": b2r if j == 0 else b2z,
            }
        )

    nc = _get_nc()
    res = run_bass_kernel_spmd(nc, in_maps, list(range(NCORES)), trace=TRACE)
    LAST_EXEC_NS = res.exec_time_ns

    parts = np.stack([res.results[j]["out"] for j in range(NCORES)])  # (8,4,32)
    ytb = parts.sum(axis=0, dtype=np.float64)                         # (4, 32)
    return np.ascontiguousarray(ytb.T.astype(np.float32))             # (B, N)
